# revision 1
# baseline (speedup 1.0000x reference)
"""Trainium2 Bass kernel for nn_MultiHeadCrossAttention (ragged kv cross-attention).

Contract: kernel(**inputs) takes FULL numpy inputs, shards across 8 NeuronCores
(data-parallel: core i handles batch i//2, query rows [(i%2)*2048, +2048)),
runs one SPMD Bass/Tile kernel, gathers the FULL output.

Per-core pipeline (all matmuls bf16, fp32 accumulation):
  P1: kT = rms-normed (kv_w_k.T @ condT), head-padded [d,h,m]; v [m,(h,73)] with
      a denominator-ones column; ragged kv handled by zeroing condT columns and
      the ones column beyond kv_seqlen (no mask add needed: zero k gives
      exp(0)=1 whose V-row contribution is 0 and whose den contribution is 0).
  P2: qT = rms-normed (x @ q_w).T, head-padded [d,h,n] via DMA-transpose;
      pipelined per 512-query block into P3.
  P3: scoresT[m,q] = kT_h.T @ qT_h; exp on ScalarE (no max-subtraction; scores
      are bounded by the rms norms); PV: out[q,(h,73)] += expT.T @ v_aug;
      normalize by the den column; DMA-transpose to attnT.
  P4 (interleaved with P3 per query block): out = attnT.T @ proj_w.
"""

import numpy as np
import ml_dtypes

import concourse.bacc as bacc
import concourse.bass as bass
import concourse.mybir as mybir
import concourse.tile as tile
from concourse.bass_utils import run_bass_kernel_spmd

BF16 = ml_dtypes.bfloat16
AF = mybir.ActivationFunctionType
ALU = mybir.AluOpType

B, N, M, C = 4, 4096, 512, 1152
H, D = 16, 72
NCORES = 8
NL = N * B // NCORES          # 2048 query rows per core
KC = C // 128                 # 9 contraction chunks
QT = NL // 128                # 16 query tiles per core
NB = NL // 512                # 4 query blocks of 512
MC = M // 128                 # 4 kv chunks
EPS = 1e-6
HB = 4                        # heads per PV group (one PSUM bank: 4*73 <= 512)
NHG = H // HB

_BUILD_CACHE = {}


def _head_ranges():
    """For head h (rows 72h..72h+72 of a chunked [128,9,*] layout), the 1-2
    (chunk, p0, dst0, rows) pieces it maps to."""
    out = []
    for h in range(H):
        a = D * h
        c0, p0 = a // 128, a % 128
        r = min(128 - p0, D)
        pieces = [(c0, p0, 0, r)]
        if r < D:
            pieces.append((c0 + 1, 0, r, D - r))
        out.append(pieces)
    return out


def _build(with_bias: bool, uniform_scale, key):
    if key in _BUILD_CACHE:
        return _BUILD_CACHE[key]

    nc = bacc.Bacc("TRN2", target_bir_lowering=False, debug=False,
                   num_devices=NCORES)
    f32, bf16 = mybir.dt.float32, mybir.dt.bfloat16

    xTt = nc.dram_tensor("xTt", [QT, 128, KC, 128], bf16, kind="ExternalInput").ap()
    condT = nc.dram_tensor("condT", [MC, 128, KC, 128], bf16, kind="ExternalInput").ap()
    qw = nc.dram_tensor("qw", [128, KC, C], bf16, kind="ExternalInput").ap()
    kvw = nc.dram_tensor("kvw", [128, KC, 2 * C], bf16, kind="ExternalInput").ap()
    pw = nc.dram_tensor("pw", [128, KC, C], bf16, kind="ExternalInput").ap()
    vones = nc.dram_tensor("vones", [M, H], bf16, kind="ExternalInput").ap()
    if uniform_scale is None:
        wk = nc.dram_tensor("wk", [128, KC], f32, kind="ExternalInput").ap()
    if with_bias:
        qb = nc.dram_tensor("qb", [1, C], bf16, kind="ExternalInput").ap()
        kvb = nc.dram_tensor("kvb", [1, 2 * C], bf16, kind="ExternalInput").ap()
        pb = nc.dram_tensor("pb", [1, C], bf16, kind="ExternalInput").ap()
        maskv = nc.dram_tensor("maskv", [1, M], bf16, kind="ExternalInput").ap()
    out = nc.dram_tensor("out", [NL, C], f32, kind="ExternalOutput").ap()

    exp_scale = float(uniform_scale) if uniform_scale is not None else 1.0
    blocks = [(0, 512), (512, 512), (1024, 128)]          # dout blocks of C
    vblocks = [(0, 7), (504, 7), (1008, 2)]               # v blocks, 73-aligned heads

    def bcast(ap_scalar, groups, width):
        # [128, groups] -> [128, groups, width] with stride-0 inner dim
        return bass.AP(tensor=ap_scalar.tensor, offset=ap_scalar.offset,
                       ap=[ap_scalar.ap[0], [ap_scalar.ap[1][0], groups], [0, width]])

    with tile.TileContext(nc) as tc:
        with tc.tile_pool(name="persist", bufs=1) as persist, \
             tc.tile_pool(name="qTnb", bufs=4) as qTnbpool, \
             tc.tile_pool(name="qTcpool", bufs=2) as qTcpool:
            kTp = persist.tile([128, H, M], bf16)          # padded kT [d<=72, h, m]
            v16 = persist.tile([128, MC, H * 73], bf16)    # v + den-ones col per head
            eps_sb = persist.tile([128, 1], f32)
            nc.vector.memset(eps_sb, EPS)
            if with_bias:
                ones1 = persist.tile([1, 128], bf16)
                nc.vector.memset(ones1, 1.0)
                qb_sb = persist.tile([1, C], bf16)
                kvb_sb = persist.tile([1, 2 * C], bf16)
                pb_sb = persist.tile([1, C], bf16)
                maskv_sb = persist.tile([1, M], bf16)
                nc.sync.dma_start(out=qb_sb, in_=qb)
                nc.sync.dma_start(out=kvb_sb, in_=kvb)
                nc.sync.dma_start(out=pb_sb, in_=pb)
                nc.sync.dma_start(out=maskv_sb, in_=maskv)
            if uniform_scale is None:
                wk_sb = persist.tile([128, KC], f32)
                nc.sync.dma_start(out=wk_sb, in_=wk)

            qTp_nbs = [None] * NB

            # ===== early scope: qw + x-stream live through P1+P2 =====
            with tc.tile_pool(name="early", bufs=1) as early, \
                 tc.tile_pool(name="p2x", bufs=2) as p2x:
                qw_sb = early.tile([128, KC, C], bf16)
                kTc = early.tile([128, KC, M], bf16)

                # ================= Phase 1: K/V projection =================
                with tc.tile_pool(name="p1sb", bufs=1) as p1sb, \
                     tc.tile_pool(name="p1small", bufs=2) as p1small, \
                     tc.tile_pool(name="p1k", bufs=2) as p1k, \
                     tc.tile_pool(name="p1psK", bufs=2, space="PSUM") as p1psK, \
                     tc.tile_pool(name="p1psV", bufs=2, space="PSUM") as p1psV:
                    kvw_sb = p1sb.tile([128, KC, 2 * C], bf16)
                    for bo in range(0, 2 * C, 512):
                        bw = min(512, 2 * C - bo)
                        nc.gpsimd.dma_start(out=kvw_sb[:, :, bo:bo + bw],
                                            in_=kvw[:, :, bo:bo + bw])
                    for bo, bw in blocks:
                        nc.gpsimd.dma_start(out=qw_sb[:, :, bo:bo + bw],
                                            in_=qw[:, :, bo:bo + bw])

                    for mc in range(MC):
                        msl = slice(mc * 128, (mc + 1) * 128)
                        condT_sb = p1k.tile([128, KC, 128], bf16, tag="ct",
                                            name=f"ct{mc}")
                        nc.sync.dma_start(out=condT_sb, in_=condT[mc])
                        psK = p1psK.tile([128, 1536], f32, tag="psK",
                                         name=f"psK{mc}")
                        for bo, bw in blocks:
                            for kc in range(KC):
                                nc.tensor.matmul(
                                    psK[:, bo:bo + bw],
                                    condT_sb[:, kc, :],
                                    kvw_sb[:, kc, bo:bo + bw],
                                    start=(kc == 0),
                                    stop=(kc == KC - 1 and not with_bias))
                            if with_bias:
                                nc.tensor.matmul(psK[:, bo:bo + bw],
                                                 maskv_sb[:, msl],
                                                 kvb_sb[:, bo:bo + bw],
                                                 start=False, stop=True)
                        ksq = p1small.tile([128, C], bf16, tag="ksq",
                                           name=f"ksq{mc}")
                        nc.scalar.activation(ksq, psK[:, 0:C], AF.Square)
                        ssqk = p1small.tile([128, H], f32, tag="ssqk",
                                            name=f"ssqk{mc}")
                        nc.vector.tensor_reduce(
                            ssqk, ksq.rearrange("p (h d) -> p h d", h=H),
                            axis=mybir.AxisListType.X, op=ALU.add)
                        lnk = p1small.tile([128, H], f32, tag="lnk",
                                           name=f"lnk{mc}")
                        nc.scalar.activation(lnk, ssqk, AF.Ln, bias=eps_sb,
                                             scale=1.0 / D)
                        invk = p1small.tile([128, H], f32, tag="invk",
                                            name=f"invk{mc}")
                        nc.scalar.activation(invk, lnk, AF.Exp, scale=-0.5)
                        k16 = p1k.tile([128, C], bf16, tag="k16", name=f"k16_{mc}")
                        nc.vector.tensor_tensor(
                            out=k16.rearrange("p (h d) -> p h d", h=H),
                            in0=psK[:, 0:C].rearrange("p (h d) -> p h d", h=H),
                            in1=bcast(invk, H, D), op=ALU.mult)
                        nc.sync.dma_start(out=kTc[:, :, msl], in_=k16,
                                          transpose=True)

                        for vo, hh in vblocks:
                            vw = hh * D
                            psV = p1psV.tile([128, 504], f32, tag="psV",
                                             name=f"psV{mc}_{vo}")
                            for kc in range(KC):
                                nc.tensor.matmul(
                                    psV[:, 0:vw],
                                    condT_sb[:, kc, :],
                                    kvw_sb[:, kc, C + vo:C + vo + vw],
                                    start=(kc == 0),
                                    stop=(kc == KC - 1 and not with_bias))
                            if with_bias:
                                nc.tensor.matmul(psV[:, 0:vw], maskv_sb[:, msl],
                                                 kvb_sb[:, C + vo:C + vo + vw],
                                                 start=False, stop=True)
                            h0 = vo // D
                            vdst = v16[:, mc, 73 * h0:73 * (h0 + hh)]
                            nc.vector.tensor_copy(
                                out=vdst.rearrange("p (h t) -> p h t",
                                                   h=hh)[:, :, 0:D],
                                in_=psV[:, 0:vw].rearrange("p (h d) -> p h d",
                                                           h=hh))
                        nc.sync.dma_start(
                            out=v16[:, mc, :].rearrange("p (h t) -> p h t",
                                                        h=H)[:, :, 72:73],
                            in_=vones[msl, :])


                    if uniform_scale is None:
                        for kc in range(KC):
                            nc.vector.tensor_scalar_mul(
                                kTc[:, kc, :], kTc[:, kc, :], wk_sb[:, kc:kc + 1])
                    for h, pieces in enumerate(_head_ranges()):
                        for (c0, p0, d0, rows) in pieces:
                            nc.sync.dma_start(out=kTp[d0:d0 + rows, h, :],
                                              in_=kTc[p0:p0 + rows, c0, :])

                # ================= Phase 2: Q projection =================
                with tc.tile_pool(name="p2q", bufs=3) as p2q, \
                     tc.tile_pool(name="p2small", bufs=3) as p2small, \
                     tc.tile_pool(name="p2ps", bufs=2, space="PSUM") as p2ps:
                    for nb in range(NB):
                        qTc = qTcpool.tile([128, KC, 512], bf16, tag="qTc",
                                         name=f"qTc{nb}")
                        qTp_nb = qTnbpool.tile([128, H, 512], bf16, tag="qTp",
                                               name=f"qTp{nb}")
                        qTp_nbs[nb] = qTp_nb
                        for j in range(4):
                            qt = nb * 4 + j
                            xt = p2x.tile([128, KC, 128], bf16, tag="xt",
                                          name=f"xt{qt}")
                            nc.gpsimd.dma_start(out=xt, in_=xTt[qt])
                            psQ = p2ps.tile([128, 1536], f32, tag="psQ",
                                            name=f"psQ{qt}")
                            for bo, bw in blocks:
                                for kc in range(KC):
                                    nc.tensor.matmul(
                                        psQ[:, bo:bo + bw], xt[:, kc, :],
                                        qw_sb[:, kc, bo:bo + bw],
                                        start=(kc == 0),
                                        stop=(kc == KC - 1 and not with_bias))
                                if with_bias:
                                    nc.tensor.matmul(psQ[:, bo:bo + bw], ones1,
                                                     qb_sb[:, bo:bo + bw],
                                                     start=False, stop=True)
                            qf32 = p2q.tile([128, C], f32, tag="qf32",
                                            name=f"qf32_{qt}")
                            nc.vector.tensor_copy(qf32, psQ[:, 0:C])
                            qsq = p2small.tile([128, C], bf16, tag="qsq",
                                               name=f"qsq{qt}")
                            nc.scalar.activation(qsq, qf32, AF.Square)
                            ssqq = p2small.tile([128, H], f32, tag="ssqq",
                                                name=f"ssqq{qt}")
                            nc.vector.tensor_reduce(
                                ssqq, qsq.rearrange("p (h d) -> p h d", h=H),
                                axis=mybir.AxisListType.X, op=ALU.add)
                            lnq = p2small.tile([128, H], f32, tag="lnq",
                                               name=f"lnq{qt}")
                            nc.scalar.activation(lnq, ssqq, AF.Ln, bias=eps_sb,
                                                 scale=1.0 / D)
                            invq = p2small.tile([128, H], f32, tag="invq",
                                                name=f"invq{qt}")
                            nc.scalar.activation(invq, lnq, AF.Exp, scale=-0.5)
                            q16 = p2q.tile([128, C], bf16, tag="q16",
                                           name=f"q16_{qt}")
                            nc.vector.tensor_tensor(
                                out=q16.rearrange("p (h d) -> p h d", h=H),
                                in0=qf32.rearrange("p (h d) -> p h d", h=H),
                                in1=bcast(invq, H, D), op=ALU.mult)
                            nc.sync.dma_start(out=qTc[:, :, j * 128:(j + 1) * 128],
                                              in_=q16, transpose=True)
                        for h, pieces in enumerate(_head_ranges()):
                            for (c0, p0, d0, rows) in pieces:
                                nc.sync.dma_start(
                                    out=qTp_nb[d0:d0 + rows, h, :],
                                    in_=qTc[p0:p0 + rows, c0, :])

            # ===== Phases 3+4: attention + output projection (interleaved) =====
            with tc.tile_pool(name="p3exp", bufs=HB + 2) as p3exp, \
                 tc.tile_pool(name="p3attn", bufs=6) as p3attn, \
                 tc.tile_pool(name="p3small", bufs=4) as p3small, \
                 tc.tile_pool(name="attnTnb", bufs=2) as attnTnb, \
                 tc.tile_pool(name="p4o", bufs=3) as p4o, \
                 tc.tile_pool(name="p3psS", bufs=2, space="PSUM") as p3psS, \
                 tc.tile_pool(name="p3psPV", bufs=2, space="PSUM") as p3psPV, \
                 tc.tile_pool(name="pwpool", bufs=1) as pwpool, \
                 tc.tile_pool(name="p4ps", bufs=2, space="PSUM") as p4ps:
                pw_sb = pwpool.tile([128, KC, C], bf16)
                for bo, bw in blocks:
                    nc.gpsimd.dma_start(out=pw_sb[:, :, bo:bo + bw],
                                        in_=pw[:, :, bo:bo + bw])

                attnTs = [None] * NB
                prev_proj = [None]   # chain proj accumulation groups only

                def proj_qt(pnb, j):
                    qt = pnb * 4 + j
                    for bo, bw in blocks:
                        psP = p4ps.tile([128, 512], f32, tag="psP",
                                        name=f"psP_{qt}_{bo}")
                        first = last = None
                        for kc in range(KC):
                            mm = nc.tensor.matmul(
                                psP[:, 0:bw],
                                attnTs[pnb][:, kc, j * 128:(j + 1) * 128],
                                pw_sb[:, kc, bo:bo + bw],
                                start=(kc == 0),
                                stop=(kc == KC - 1 and not with_bias))
                            if first is None:
                                first = mm
                            last = mm
                        if with_bias:
                            last = nc.tensor.matmul(psP[:, 0:bw], ones1,
                                                    pb_sb[:, bo:bo + bw],
                                                    start=False, stop=True)
                        if prev_proj[0] is not None:
                            tile.add_dep_helper(first.ins, prev_proj[0].ins,
                                                sync=False,
                                                reason="proj psum group order")
                        prev_proj[0] = last
                        so = p4o.tile([128, 512], f32, tag="so",
                                      name=f"so_{qt}_{bo}")
                        nc.vector.tensor_copy(so[:, 0:bw], psP[:, 0:bw])
                        nc.sync.dma_start(
                            out=out[qt * 128:(qt + 1) * 128, bo:bo + bw],
                            in_=so[:, 0:bw])

                for nb in range(NB):
                    qTp_nb = qTp_nbs[nb]
                    attnT = attnTnb.tile([128, KC, 512], bf16, tag="attnT",
                                         name=f"attnT{nb}")
                    attnTs[nb] = attnT
                    attnN = [p3attn.tile([128, C], bf16, tag="attnN",
                                         name=f"attnN_{nb}_{j}")
                             for j in range(4)]
                    for hg in range(NHG):
                        if nb > 0:
                            proj_qt(nb - 1, hg)
                        expts = []
                        for hl in range(HB):
                            h = hg * HB + hl
                            expt = p3exp.tile([128, 2048], bf16, tag="expt",
                                              name=f"expt_{nb}_{h}")
                            for half in range(2):
                                psS = p3psS.tile([128, 1024], f32, tag="psS",
                                                 name=f"psS_{nb}_{h}_{half}")
                                for mcl in range(2):
                                    mc = half * 2 + mcl
                                    nc.tensor.matmul(
                                        psS[:, mcl * 512:(mcl + 1) * 512],
                                        kTp[0:D, h, mc * 128:(mc + 1) * 128],
                                        qTp_nb[0:D, h, :],
                                        start=True, stop=True)
                                nc.scalar.activation(
                                    expt[:, half * 1024:(half + 1) * 1024],
                                    psS, AF.Exp, scale=exp_scale)
                            expts.append(expt)
                        for j in range(4):
                            # HB heads share one PSUM bank; accumulation groups
                            # are chained in emission order (start=True clears
                            # the whole bank's has_written bits).
                            psPV = p3psPV.tile([128, 512], f32, tag="psPV",
                                               name=f"psPV_{nb}_{hg}_{j}")
                            prev_last = None
                            for hl in range(HB):
                                h = hg * HB + hl
                                first = last = None
                                for mc in range(MC):
                                    mm = nc.tensor.matmul(
                                        psPV[:, hl * 73:(hl + 1) * 73],
                                        expts[hl][:, mc * 512 + j * 128:
                                                  mc * 512 + (j + 1) * 128],
                                        v16[:, mc, 73 * h:73 * h + 73],
                                        start=(mc == 0), stop=(mc == MC - 1))
                                    if first is None:
                                        first = mm
                                    last = mm
                                if prev_last is not None:
                                    tile.add_dep_helper(
                                        first.ins, prev_last.ins, sync=False,
                                        reason="psum-bank accum group order")
                                prev_last = last
                            dens = p3small.tile([128, HB], f32, tag="dens",
                                                name=f"dens_{nb}_{hg}_{j}")
                            pv3 = bass.AP(
                                tensor=psPV.tensor, offset=psPV.offset,
                                ap=[psPV.ap[0], [73, HB], [1, 73]])
                            nc.vector.tensor_copy(out=dens, in_=pv3[:, :, 72])
                            rec = p3small.tile([128, HB], f32, tag="rec",
                                               name=f"rec_{nb}_{hg}_{j}")
                            nc.vector.reciprocal(rec, dens)
                            nc.vector.tensor_tensor(
                                out=attnN[j][:, hg * HB * D:(hg + 1) * HB * D]
                                    .rearrange("p (h d) -> p h d", h=HB),
                                in0=pv3[:, :, 0:D],
                                in1=bcast(rec, HB, D), op=ALU.mult)
                    for j in range(4):
                        nc.sync.dma_start(
                            out=attnT[:, :, j * 128:(j + 1) * 128],
                            in_=attnN[j], transpose=True)
                    if nb == NB - 1:
                        for j in range(4):
                            proj_qt(nb, j)

    nc.compile()
    _BUILD_CACHE[key] = nc
    return nc


def kernel(x, cond, kv_seqlen, q_w, q_b, kv_w, kv_b, proj_w, proj_b, qn_w, kn_w):
    x = np.asarray(x); cond = np.asarray(cond)
    kv_seqlen = np.asarray(kv_seqlen)
    q_w = np.asarray(q_w, np.float32); q_b = np.asarray(q_b, np.float32)
    kv_w = np.asarray(kv_w, np.float32); kv_b = np.asarray(kv_b, np.float32)
    proj_w = np.asarray(proj_w, np.float32); proj_b = np.asarray(proj_b, np.float32)
    qn_w = np.asarray(qn_w, np.float32); kn_w = np.asarray(kn_w, np.float32)

    with_bias = bool(np.any(q_b) or np.any(kv_b) or np.any(proj_b))
    qk = (qn_w * kn_w).astype(np.float64)
    if np.all(qk == qk[0]):
        uniform_scale = float(qk[0]) / float(np.sqrt(D))
    else:
        uniform_scale = None
    key = (with_bias, uniform_scale)
    nc = _build(with_bias, uniform_scale, key)

    def blocked_w(w):  # [C, dout] -> [128, KC, dout]
        return np.ascontiguousarray(
            w.reshape(KC, 128, -1).transpose(1, 0, 2)).astype(BF16)

    qwb = blocked_w(q_w)
    kvwb = blocked_w(kv_w)
    pwb = blocked_w(proj_w)
    if uniform_scale is None:
        wk_flat = np.tile(qn_w * kn_w, H).astype(np.float32) / np.sqrt(D)
        wkb = np.ascontiguousarray(wk_flat.reshape(KC, 128).T).astype(np.float32)

    in_maps = []
    for core in range(NCORES):
        b, ns = core // 2, (core % 2) * NL
        A = x[b, ns:ns + NL, :].astype(np.float32)
        xtt = np.ascontiguousarray(
            A.reshape(QT, 128, KC, 128).transpose(0, 3, 2, 1)).astype(BF16)
        sl = int(kv_seqlen[b])
        ct = cond[b].astype(np.float32).T.copy()       # [C, M]
        ct[:, sl:] = 0.0
        ctb = np.ascontiguousarray(
            ct.reshape(KC, 128, MC, 128).transpose(2, 1, 0, 3)).astype(BF16)
        valid = (np.arange(M) < sl)
        vob = np.ascontiguousarray(
            np.repeat(valid[:, None], H, axis=1)).astype(BF16)
        m = {"xTt": xtt, "condT": ctb, "qw": qwb, "kvw": kvwb, "pw": pwb,
             "vones": vob}
        if uniform_scale is None:
            m["wk"] = wkb
        if with_bias:
            m["qb"] = q_b[None, :].astype(BF16)
            m["kvb"] = kv_b[None, :].astype(BF16)
            m["pb"] = proj_b[None, :].astype(BF16)
            m["maskv"] = valid[None, :].astype(BF16)
        in_maps.append(m)

    res = run_bass_kernel_spmd(nc, in_maps, core_ids=list(range(NCORES)))
    kernel._last_results = res

    out = np.empty((B, N, C), np.float32)
    for core in range(NCORES):
        b, ns = core // 2, (core % 2) * NL
        out[b, ns:ns + NL, :] = res.results[core]["out"]
    return out



# revision 14
# speedup vs baseline: 1.0307x; 1.0307x over previous
"""Trainium2 Bass kernel for nn_MultiHeadCrossAttention (ragged kv cross-attention).

Contract: kernel(**inputs) takes FULL numpy inputs, shards across 8 NeuronCores
(data-parallel: core i handles batch i//2, query rows [(i%2)*2048, +2048)),
runs one SPMD Bass/Tile kernel, gathers the FULL output.

Per-core pipeline (all matmuls bf16, fp32 accumulation):
  P1: kT = rms-normed (kv_w_k.T @ condT), head-padded [d,h,m] via one
      padded-source DMA transpose per kv chunk; v [m,(h,73)] with a
      denominator-ones column; ragged kv handled by zeroing condT columns and
      the ones column beyond kv_seqlen (no mask add needed: zero k gives
      exp(0)=1 whose V-row contribution is 0 and whose den contribution is 0).
  P2: qT = rms-normed (x @ q_w).T, head-padded [d,h,n] via one padded-source
      DMA transpose per 128-row tile; rms ops read PSUM directly.
  P3: scoresT[m,q] = kT_h.T @ qT_h; exp on ScalarE (no max-subtraction; scores
      are bounded by the rms norms); PV: out[q,(h,73)] += expT.T @ v_aug;
      normalize by the den column; DMA-transpose to attnT.
  P4 (interleaved with P3 per query block): out = attnT.T @ proj_w.

The rsqrt in rms-norm is ACT Sqrt + DVE reciprocal (not Ln/Exp) so ScalarE
needs only two activation-table loads for the whole kernel (sqrt set for
P1/P2, exp set for P3).
"""

import numpy as np
import ml_dtypes

import concourse.bacc as bacc
import concourse.bass as bass
import concourse.mybir as mybir
import concourse.tile as tile
from concourse.bass_utils import run_bass_kernel_spmd

BF16 = ml_dtypes.bfloat16
AF = mybir.ActivationFunctionType
ALU = mybir.AluOpType

B, N, M, C = 4, 4096, 512, 1152
H, D = 16, 72
NCORES = 8
NL = N * B // NCORES          # 2048 query rows per core
KC = C // 128                 # 9 contraction chunks
QT = NL // 128                # 16 query tiles per core
NB = NL // 512                # 4 query blocks of 512
MC = M // 128                 # 4 kv chunks
EPS = 1e-6
HB = 4                        # heads per PV group (one PSUM bank: 4*73 <= 512)
NHG = H // HB

_BUILD_CACHE = {}


def _build(with_bias: bool, uniform_scale, key):
    if key in _BUILD_CACHE:
        return _BUILD_CACHE[key]

    nc = bacc.Bacc("TRN2", target_bir_lowering=False, debug=False,
                   num_devices=NCORES)
    f32, bf16 = mybir.dt.float32, mybir.dt.bfloat16

    xTt = nc.dram_tensor("xTt", [QT, 128, KC, 128], bf16, kind="ExternalInput").ap()
    condT = nc.dram_tensor("condT", [MC, 128, KC, 128], bf16, kind="ExternalInput").ap()
    qw = nc.dram_tensor("qw", [128, KC, C], bf16, kind="ExternalInput").ap()
    kvw = nc.dram_tensor("kvw", [128, KC, 2 * C], bf16, kind="ExternalInput").ap()
    pw = nc.dram_tensor("pw", [128, KC, C], bf16, kind="ExternalInput").ap()
    vones = nc.dram_tensor("vones", [M, H], bf16, kind="ExternalInput").ap()
    if uniform_scale is None:
        # per-(h,d) q/k norm-weight product, replicated over partitions and
        # laid out in the head-padded [h*128+d] column space
        wk = nc.dram_tensor("wk", [128, H * 128], bf16, kind="ExternalInput").ap()
    if with_bias:
        qb = nc.dram_tensor("qb", [1, C], bf16, kind="ExternalInput").ap()
        kvb = nc.dram_tensor("kvb", [1, 2 * C], bf16, kind="ExternalInput").ap()
        pb = nc.dram_tensor("pb", [1, C], bf16, kind="ExternalInput").ap()
        maskv = nc.dram_tensor("maskv", [1, M], bf16, kind="ExternalInput").ap()
    out = nc.dram_tensor("out", [NL, C], f32, kind="ExternalOutput").ap()

    exp_scale = float(uniform_scale) if uniform_scale is not None else 1.0
    blocks = [(0, 512), (512, 512), (1024, 128)]          # dout blocks of C
    vblocks = [(0, 7), (504, 7), (1008, 2)]               # v blocks, 73-aligned heads

    def bcast(ap_scalar, groups, width):
        # [128, groups] -> [128, groups, width] with stride-0 inner dim
        return bass.AP(tensor=ap_scalar.tensor, offset=ap_scalar.offset,
                       ap=[ap_scalar.ap[0], [ap_scalar.ap[1][0], groups], [0, width]])

    with tile.TileContext(nc) as tc:
        with tc.tile_pool(name="persist", bufs=1) as persist, \
             tc.tile_pool(name="qTnb", bufs=2) as qTnbpool:
            kTp = persist.tile([128, H, M], bf16)          # padded kT [d<=72, h, m]
            v16 = persist.tile([128, MC, H * 73], bf16)    # v + den-ones col per head
            eps_sb = persist.tile([128, 1], f32)
            nc.vector.memset(eps_sb, EPS)
            if with_bias:
                ones1 = persist.tile([1, 128], bf16)
                nc.vector.memset(ones1, 1.0)
                qb_sb = persist.tile([1, C], bf16)
                kvb_sb = persist.tile([1, 2 * C], bf16)
                pb_sb = persist.tile([1, C], bf16)
                maskv_sb = persist.tile([1, M], bf16)
                nc.sync.dma_start(out=qb_sb, in_=qb)
                nc.sync.dma_start(out=kvb_sb, in_=kvb)
                nc.sync.dma_start(out=pb_sb, in_=pb)
                nc.sync.dma_start(out=maskv_sb, in_=maskv)
            if uniform_scale is None:
                wk_sb = persist.tile([128, H, 128], bf16)
                nc.sync.dma_start(out=wk_sb.rearrange("p h d -> p (h d)"), in_=wk)

            qTp_nbs = [None] * NB

            # ===== early scope: qw + x-stream live through P1+P2 =====
            with tc.tile_pool(name="early", bufs=1) as early, \
                 tc.tile_pool(name="p2x", bufs=3) as p2x:
                qw_sb = early.tile([128, KC, C], bf16)

                # ================= Phase 1: K/V projection =================
                with tc.tile_pool(name="p1sb", bufs=1) as p1sb, \
                     tc.tile_pool(name="p1small", bufs=2) as p1small, \
                     tc.tile_pool(name="p1k", bufs=2) as p1k, \
                     tc.tile_pool(name="p1psK", bufs=2, space="PSUM") as p1psK, \
                     tc.tile_pool(name="p1psV", bufs=2, space="PSUM") as p1psV:
                    kvw_sb = p1sb.tile([128, KC, 2 * C], bf16)
                    for bo in range(0, 2 * C, 512):
                        bw = min(512, 2 * C - bo)
                        nc.gpsimd.dma_start(out=kvw_sb[:, :, bo:bo + bw],
                                            in_=kvw[:, :, bo:bo + bw])
                    for bo, bw in blocks:
                        nc.gpsimd.dma_start(out=qw_sb[:, :, bo:bo + bw],
                                            in_=qw[:, :, bo:bo + bw])

                    kpend = [None]

                    def k_tail():
                        if kpend[0] is None:
                            return
                        psK, ssqk, mc = kpend[0]
                        kpend[0] = None
                        msl = slice(mc * 128, (mc + 1) * 128)
                        sdk = p1small.tile([128, H], f32, tag="sdk",
                                           name=f"sdk{mc}")
                        nc.scalar.activation(sdk, ssqk, AF.Sqrt, bias=eps_sb,
                                             scale=1.0 / D)
                        invk = p1small.tile([128, H], f32, tag="invk",
                                            name=f"invk{mc}")
                        nc.vector.reciprocal(invk, sdk)
                        # head-padded row layout: head h at cols [128h, 128h+72)
                        k16p = p1k.tile([128, H, 128], bf16, tag="k16",
                                        name=f"k16_{mc}")
                        nc.vector.tensor_tensor(
                            out=k16p[:, :, 0:D],
                            in0=psK[:, 0:C].rearrange("p (h d) -> p h d", h=H),
                            in1=bcast(invk, H, D), op=ALU.mult)
                        if uniform_scale is None:
                            nc.vector.tensor_tensor(
                                out=k16p[:, :, 0:D], in0=k16p[:, :, 0:D],
                                in1=wk_sb[:, :, 0:D], op=ALU.mult)
                        nc.sync.dma_start(
                            out=kTp[:, :, msl],
                            in_=k16p.rearrange("p h d -> p (h d)"),
                            transpose=True)

                    for mc in range(MC):
                        msl = slice(mc * 128, (mc + 1) * 128)
                        condT_sb = p1k.tile([128, KC, 128], bf16, tag="ct",
                                            name=f"ct{mc}")
                        nc.sync.dma_start(out=condT_sb, in_=condT[mc])
                        psK = p1psK.tile([128, 1536], f32, tag="psK",
                                         name=f"psK{mc}")
                        for bo, bw in blocks:
                            for kc in range(KC):
                                nc.tensor.matmul(
                                    psK[:, bo:bo + bw],
                                    condT_sb[:, kc, :],
                                    kvw_sb[:, kc, bo:bo + bw],
                                    start=(kc == 0),
                                    stop=(kc == KC - 1 and not with_bias))
                            if with_bias:
                                nc.tensor.matmul(psK[:, bo:bo + bw],
                                                 maskv_sb[:, msl],
                                                 kvb_sb[:, bo:bo + bw],
                                                 start=False, stop=True)
                        ksq = p1small.tile([128, C], bf16, tag="ksq",
                                           name=f"ksq{mc}")
                        nc.scalar.activation(ksq, psK[:, 0:C], AF.Square)
                        ssqk = p1small.tile([128, H], f32, tag="ssqk",
                                            name=f"ssqk{mc}")
                        nc.vector.tensor_reduce(
                            ssqk, ksq.rearrange("p (h d) -> p h d", h=H),
                            axis=mybir.AxisListType.X, op=ALU.add)
                        k_tail()
                        kpend[0] = (psK, ssqk, mc)

                        for vo, hh in vblocks:
                            vw = hh * D
                            psV = p1psV.tile([128, 504], f32, tag="psV",
                                             name=f"psV{mc}_{vo}")
                            for kc in range(KC):
                                nc.tensor.matmul(
                                    psV[:, 0:vw],
                                    condT_sb[:, kc, :],
                                    kvw_sb[:, kc, C + vo:C + vo + vw],
                                    start=(kc == 0),
                                    stop=(kc == KC - 1 and not with_bias))
                            if with_bias:
                                nc.tensor.matmul(psV[:, 0:vw], maskv_sb[:, msl],
                                                 kvb_sb[:, C + vo:C + vo + vw],
                                                 start=False, stop=True)
                            h0 = vo // D
                            vdst = v16[:, mc, 73 * h0:73 * (h0 + hh)]
                            nc.vector.tensor_copy(
                                out=vdst.rearrange("p (h t) -> p h t",
                                                   h=hh)[:, :, 0:D],
                                in_=psV[:, 0:vw].rearrange("p (h d) -> p h d",
                                                           h=hh))
                        nc.sync.dma_start(
                            out=v16[:, mc, :].rearrange("p (h t) -> p h t",
                                                        h=H)[:, :, 72:73],
                            in_=vones[msl, :])
                    k_tail()

                # ================= Phase 2a: Q projection =================
                # Per tile: matmuls -> ACT copy -> ACT square -> DVE per-head
                # reduce into one shared ssq tile. The rsqrt is NOT computed
                # per tile: one batched Sqrt + reciprocal covers all 16 tiles
                # after the loop, so no ACT queue-head op ever waits on a
                # same-tile cross-engine result and the PE stream never
                # stalls on the rms chain.
                with tc.tile_pool(name="p2small", bufs=3) as p2small, \
                     tc.tile_pool(name="p2ps", bufs=2, space="PSUM") as p2ps:
                    ssq_all = persist.tile([128, QT, H], f32)
                    inv_all = persist.tile([128, QT, H], f32)
                    qcps = [None] * QT
                    for qt in range(QT):
                        xt = p2x.tile([128, KC, 128], bf16, tag="xt",
                                      name=f"xt{qt}")
                        nc.gpsimd.dma_start(out=xt, in_=xTt[qt])
                        psQ = p2ps.tile([128, 1536], f32, tag="psQ",
                                        name=f"psQ{qt}")
                        for bo, bw in blocks:
                            for kc in range(KC):
                                nc.tensor.matmul(
                                    psQ[:, bo:bo + bw], xt[:, kc, :],
                                    qw_sb[:, kc, bo:bo + bw],
                                    start=(kc == 0),
                                    stop=(kc == KC - 1 and not with_bias))
                            if with_bias:
                                nc.tensor.matmul(psQ[:, bo:bo + bw], ones1,
                                                 qb_sb[:, bo:bo + bw],
                                                 start=False, stop=True)
                        qcp = persist.tile([128, C], bf16, name=f"qcp{qt}")
                        qcps[qt] = qcp
                        nc.scalar.activation(qcp, psQ[:, 0:C], AF.Copy)
                        qsq = p2small.tile([128, C], bf16, tag="qsq",
                                           name=f"qsq{qt}")
                        nc.scalar.activation(qsq, qcp, AF.Square)
                        nc.vector.tensor_reduce(
                            ssq_all[:, qt, :],
                            qsq.rearrange("p (h d) -> p h d", h=H),
                            axis=mybir.AxisListType.X, op=ALU.add)
                    sd_all = persist.tile([128, QT, H], f32)
                    nc.scalar.activation(
                        sd_all.rearrange("p t h -> p (t h)"),
                        ssq_all.rearrange("p t h -> p (t h)"),
                        AF.Sqrt, bias=eps_sb, scale=1.0 / D)
                    nc.vector.reciprocal(
                        inv_all.rearrange("p t h -> p (t h)"),
                        sd_all.rearrange("p t h -> p (t h)"))

            # ===== Phases 3+4: attention + output projection (interleaved) =====
            with tc.tile_pool(name="p2q", bufs=3) as p2q, \
                 tc.tile_pool(name="p3exp", bufs=HB + 2) as p3exp, \
                 tc.tile_pool(name="p3attn", bufs=6) as p3attn, \
                 tc.tile_pool(name="p3small", bufs=4) as p3small, \
                 tc.tile_pool(name="attnTnb", bufs=2) as attnTnb, \
                 tc.tile_pool(name="p4o", bufs=2) as p4o, \
                 tc.tile_pool(name="p3psS", bufs=2, space="PSUM") as p3psS, \
                 tc.tile_pool(name="p3psPV", bufs=2, space="PSUM") as p3psPV, \
                 tc.tile_pool(name="pwpool", bufs=1) as pwpool, \
                 tc.tile_pool(name="p4ps", bufs=2, space="PSUM") as p4ps:
                pw_sb = pwpool.tile([128, KC, C], bf16)
                for bo, bw in blocks:
                    nc.gpsimd.dma_start(out=pw_sb[:, :, bo:bo + bw],
                                        in_=pw[:, :, bo:bo + bw])

                # ==== Phase 2b: scale + head-padded transpose, per block ====
                def q_finish(nb):
                    qTp_nb = qTnbpool.tile([128, H, 512], bf16, tag="qTp",
                                           name=f"qTp{nb}")
                    qTp_nbs[nb] = qTp_nb
                    for j in range(4):
                        qt = nb * 4 + j
                        q16p = p2q.tile([128, H, 128], bf16, tag="q16",
                                        name=f"q16_{qt}")
                        nc.vector.tensor_tensor(
                            out=q16p[:, :, 0:D],
                            in0=qcps[qt].rearrange("p (h d) -> p h d", h=H),
                            in1=bcast(inv_all[:, qt, :], H, D), op=ALU.mult)
                        nc.sync.dma_start(
                            out=qTp_nb[:, :, j * 128:(j + 1) * 128],
                            in_=q16p.rearrange("p h d -> p (h d)"),
                            transpose=True)

                q_finish(0)
                q_finish(1)

                attnTs = [None] * NB
                prev_proj = [None]   # chain proj accumulation groups only

                def proj_qt_blocks(pnb, j):
                    """Yield per-block emitters for one 128-row output tile,
                    so proj matmuls can interleave between score emissions."""
                    qt = pnb * 4 + j
                    so = p4o.tile([128, C], f32, tag="so", name=f"so_{qt}")

                    def emit_block(bo, bw, is_last):
                        psP = p4ps.tile([128, 512], f32, tag="psP",
                                        name=f"psP_{qt}_{bo}")
                        first = last = None
                        for kc in range(KC):
                            mm = nc.tensor.matmul(
                                psP[:, 0:bw],
                                attnTs[pnb][:, kc, j * 128:(j + 1) * 128],
                                pw_sb[:, kc, bo:bo + bw],
                                start=(kc == 0),
                                stop=(kc == KC - 1 and not with_bias))
                            if first is None:
                                first = mm
                            last = mm
                        if with_bias:
                            last = nc.tensor.matmul(psP[:, 0:bw], ones1,
                                                    pb_sb[:, bo:bo + bw],
                                                    start=False, stop=True)
                        if prev_proj[0] is not None:
                            tile.add_dep_helper(first.ins, prev_proj[0].ins,
                                                sync=False,
                                                reason="proj psum group order")
                        prev_proj[0] = last
                        nc.vector.tensor_copy(so[:, bo:bo + bw], psP[:, 0:bw])
                        if is_last:
                            nc.sync.dma_start(
                                out=out[qt * 128:(qt + 1) * 128, :], in_=so)

                    return [lambda bo=bo, bw=bw, lastb=(i == len(blocks) - 1):
                            emit_block(bo, bw, lastb)
                            for i, (bo, bw) in enumerate(blocks)]

                def proj_qt(pnb, j):
                    for emit in proj_qt_blocks(pnb, j):
                        emit()

                for nb in range(NB):
                    if nb + 2 < NB:
                        q_finish(nb + 2)
                    qTp_nb = qTp_nbs[nb]
                    attnT = attnTnb.tile([128, KC, 512], bf16, tag="attnT",
                                         name=f"attnT{nb}")
                    attnTs[nb] = attnT
                    attnN = [p3attn.tile([128, C], bf16, tag="attnN",
                                         name=f"attnN_{nb}_{j}")
                             for j in range(4)]
                    for hg in range(NHG):
                        pblocks = (proj_qt_blocks(nb - 1, hg) if nb > 0
                                   else [None] * 3)
                        expts = []
                        for hl in range(HB):
                            h = hg * HB + hl
                            expt = p3exp.tile([128, 2048], bf16, tag="expt",
                                              name=f"expt_{nb}_{h}")
                            for half in range(2):
                                psS = p3psS.tile([128, 1024], f32, tag="psS",
                                                 name=f"psS_{nb}_{h}_{half}")
                                for mcl in range(2):
                                    mc = half * 2 + mcl
                                    nc.tensor.matmul(
                                        psS[:, mcl * 512:(mcl + 1) * 512],
                                        kTp[0:D, h, mc * 128:(mc + 1) * 128],
                                        qTp_nb[0:D, h, :],
                                        start=True, stop=True)
                                nc.scalar.activation(
                                    expt[:, half * 1024:(half + 1) * 1024],
                                    psS, AF.Exp, scale=exp_scale)
                            expts.append(expt)
                            # fill the exp-wait PE bubble with a proj block
                            if hl >= 1 and pblocks[hl - 1] is not None:
                                pblocks[hl - 1]()
                        for j in range(4):
                            # HB heads share one PSUM bank; accumulation groups
                            # are chained in emission order (start=True clears
                            # the whole bank's has_written bits).
                            psPV = p3psPV.tile([128, 512], f32, tag="psPV",
                                               name=f"psPV_{nb}_{hg}_{j}")
                            prev_last = None
                            for hl in range(HB):
                                h = hg * HB + hl
                                first = last = None
                                for mc in range(MC):
                                    mm = nc.tensor.matmul(
                                        psPV[:, hl * 73:(hl + 1) * 73],
                                        expts[hl][:, mc * 512 + j * 128:
                                                  mc * 512 + (j + 1) * 128],
                                        v16[:, mc, 73 * h:73 * h + 73],
                                        start=(mc == 0), stop=(mc == MC - 1))
                                    if first is None:
                                        first = mm
                                    last = mm
                                if prev_last is not None:
                                    tile.add_dep_helper(
                                        first.ins, prev_last.ins, sync=False,
                                        reason="psum-bank accum group order")
                                prev_last = last
                            dens = p3small.tile([128, HB], f32, tag="dens",
                                                name=f"dens_{nb}_{hg}_{j}")
                            pv3 = bass.AP(
                                tensor=psPV.tensor, offset=psPV.offset,
                                ap=[psPV.ap[0], [73, HB], [1, 73]])
                            nc.vector.tensor_copy(out=dens, in_=pv3[:, :, 72])
                            rec = p3small.tile([128, HB], f32, tag="rec",
                                               name=f"rec_{nb}_{hg}_{j}")
                            nc.vector.reciprocal(rec, dens)
                            nc.vector.tensor_tensor(
                                out=attnN[j][:, hg * HB * D:(hg + 1) * HB * D]
                                    .rearrange("p (h d) -> p h d", h=HB),
                                in0=pv3[:, :, 0:D],
                                in1=bcast(rec, HB, D), op=ALU.mult)
                    for j in range(4):
                        nc.sync.dma_start(
                            out=attnT[:, :, j * 128:(j + 1) * 128],
                            in_=attnN[j], transpose=True)
                    if nb == NB - 1:
                        for j in range(4):
                            proj_qt(nb, j)

    nc.compile()
    _BUILD_CACHE[key] = nc
    return nc


def kernel(x, cond, kv_seqlen, q_w, q_b, kv_w, kv_b, proj_w, proj_b, qn_w, kn_w):
    x = np.asarray(x); cond = np.asarray(cond)
    kv_seqlen = np.asarray(kv_seqlen)
    q_w = np.asarray(q_w, np.float32); q_b = np.asarray(q_b, np.float32)
    kv_w = np.asarray(kv_w, np.float32); kv_b = np.asarray(kv_b, np.float32)
    proj_w = np.asarray(proj_w, np.float32); proj_b = np.asarray(proj_b, np.float32)
    qn_w = np.asarray(qn_w, np.float32); kn_w = np.asarray(kn_w, np.float32)

    with_bias = bool(np.any(q_b) or np.any(kv_b) or np.any(proj_b))
    qk = (qn_w * kn_w).astype(np.float64)
    if np.all(qk == qk[0]):
        uniform_scale = float(qk[0]) / float(np.sqrt(D))
    else:
        uniform_scale = None
    key = (with_bias, uniform_scale)
    nc = _build(with_bias, uniform_scale, key)

    def blocked_w(w):  # [C, dout] -> [128, KC, dout]
        return np.ascontiguousarray(
            w.reshape(KC, 128, -1).transpose(1, 0, 2)).astype(BF16)

    qwb = blocked_w(q_w)
    kvwb = blocked_w(kv_w)
    pwb = blocked_w(proj_w)
    if uniform_scale is None:
        wk_pad = np.zeros((H, 128), np.float32)
        wk_pad[:, 0:D] = (qn_w * kn_w)[None, :] / np.sqrt(D)
        wkb = np.ascontiguousarray(
            np.broadcast_to(wk_pad.reshape(1, H * 128),
                            (128, H * 128))).astype(BF16)

    in_maps = []
    for core in range(NCORES):
        b, ns = core // 2, (core % 2) * NL
        A = x[b, ns:ns + NL, :].astype(np.float32)
        xtt = np.ascontiguousarray(
            A.reshape(QT, 128, KC, 128).transpose(0, 3, 2, 1)).astype(BF16)
        sl = int(kv_seqlen[b])
        ct = cond[b].astype(np.float32).T.copy()       # [C, M]
        ct[:, sl:] = 0.0
        ctb = np.ascontiguousarray(
            ct.reshape(KC, 128, MC, 128).transpose(2, 1, 0, 3)).astype(BF16)
        valid = (np.arange(M) < sl)
        vob = np.ascontiguousarray(
            np.repeat(valid[:, None], H, axis=1)).astype(BF16)
        m = {"xTt": xtt, "condT": ctb, "qw": qwb, "kvw": kvwb, "pw": pwb,
             "vones": vob}
        if uniform_scale is None:
            m["wk"] = wkb
        if with_bias:
            m["qb"] = q_b[None, :].astype(BF16)
            m["kvb"] = kv_b[None, :].astype(BF16)
            m["pb"] = proj_b[None, :].astype(BF16)
            m["maskv"] = valid[None, :].astype(BF16)
        in_maps.append(m)

    res = run_bass_kernel_spmd(nc, in_maps, core_ids=list(range(NCORES)))
    kernel._last_results = res

    out = np.empty((B, N, C), np.float32)
    for core in range(NCORES):
        b, ns = core // 2, (core % 2) * NL
        out[b, ns:ns + NL, :] = res.results[core]["out"]
    return out


# revision 15
# speedup vs baseline: 1.0402x; 1.0092x over previous
"""Trainium2 Bass kernel for nn_MultiHeadCrossAttention (ragged kv cross-attention).

Contract: kernel(**inputs) takes FULL numpy inputs, shards across 8 NeuronCores
(data-parallel: core i handles batch i//2, query rows [(i%2)*2048, +2048)),
runs one SPMD Bass/Tile kernel, gathers the FULL output.

Per-core pipeline (all matmuls bf16, fp32 accumulation):
  P1: kT = rms-normed (kv_w_k.T @ condT), head-padded [d,h,m] via one
      padded-source DMA transpose per kv chunk; v [m,(h,73)] with a
      denominator-ones column; ragged kv handled by zeroing condT columns and
      the ones column beyond kv_seqlen (no mask add needed: zero k gives
      exp(0)=1 whose V-row contribution is 0 and whose den contribution is 0).
  P2: qT = rms-normed (x @ q_w).T, head-padded [d,h,n] via one padded-source
      DMA transpose per 128-row tile; rms ops read PSUM directly.
  P3: scoresT[m,q] = kT_h.T @ qT_h; exp on ScalarE (no max-subtraction; scores
      are bounded by the rms norms); PV: out[q,(h,73)] += expT.T @ v_aug;
      normalize by the den column; DMA-transpose to attnT.
  P4 (interleaved with P3 per query block): out = attnT.T @ proj_w.

The rsqrt in rms-norm is ACT Sqrt + DVE reciprocal (not Ln/Exp) so ScalarE
needs only two activation-table loads for the whole kernel (sqrt set for
P1/P2, exp set for P3).
"""

import numpy as np
import ml_dtypes

import concourse.bacc as bacc
import concourse.bass as bass
import concourse.mybir as mybir
import concourse.tile as tile
from concourse.bass_utils import run_bass_kernel_spmd

BF16 = ml_dtypes.bfloat16
AF = mybir.ActivationFunctionType
ALU = mybir.AluOpType

B, N, M, C = 4, 4096, 512, 1152
H, D = 16, 72
NCORES = 8
NL = N * B // NCORES          # 2048 query rows per core
KC = C // 128                 # 9 contraction chunks
QT = NL // 128                # 16 query tiles per core
NB = NL // 512                # 4 query blocks of 512
MC = M // 128                 # 4 kv chunks
EPS = 1e-6
HB = 4                        # heads per PV group (one PSUM bank: 4*73 <= 512)
NHG = H // HB

_BUILD_CACHE = {}


def _build(with_bias: bool, uniform_scale, key):
    if key in _BUILD_CACHE:
        return _BUILD_CACHE[key]

    nc = bacc.Bacc("TRN2", target_bir_lowering=False, debug=False,
                   num_devices=NCORES)
    f32, bf16 = mybir.dt.float32, mybir.dt.bfloat16

    xTt = nc.dram_tensor("xTt", [QT, 128, KC, 128], bf16, kind="ExternalInput").ap()
    condT = nc.dram_tensor("condT", [MC, 128, KC, 128], bf16, kind="ExternalInput").ap()
    qw = nc.dram_tensor("qw", [128, KC, C], bf16, kind="ExternalInput").ap()
    kvw = nc.dram_tensor("kvw", [128, KC, 2 * C], bf16, kind="ExternalInput").ap()
    pw = nc.dram_tensor("pw", [128, KC, C], bf16, kind="ExternalInput").ap()
    vones = nc.dram_tensor("vones", [M, H], bf16, kind="ExternalInput").ap()
    if uniform_scale is None:
        # per-(h,d) q/k norm-weight product, replicated over partitions and
        # laid out in the head-padded [h*128+d] column space
        wk = nc.dram_tensor("wk", [128, H * 128], bf16, kind="ExternalInput").ap()
    if with_bias:
        qb = nc.dram_tensor("qb", [1, C], bf16, kind="ExternalInput").ap()
        kvb = nc.dram_tensor("kvb", [1, 2 * C], bf16, kind="ExternalInput").ap()
        pb = nc.dram_tensor("pb", [1, C], bf16, kind="ExternalInput").ap()
        maskv = nc.dram_tensor("maskv", [1, M], bf16, kind="ExternalInput").ap()
    out = nc.dram_tensor("out", [NL, C], f32, kind="ExternalOutput").ap()

    exp_scale = float(uniform_scale) if uniform_scale is not None else 1.0
    blocks = [(0, 512), (512, 512), (1024, 128)]          # dout blocks of C
    vblocks = [(0, 7), (504, 7), (1008, 2)]               # v blocks, 73-aligned heads

    def bcast(ap_scalar, groups, width):
        # [128, groups] -> [128, groups, width] with stride-0 inner dim
        return bass.AP(tensor=ap_scalar.tensor, offset=ap_scalar.offset,
                       ap=[ap_scalar.ap[0], [ap_scalar.ap[1][0], groups], [0, width]])

    with tile.TileContext(nc) as tc:
        with tc.tile_pool(name="persist", bufs=1) as persist, \
             tc.tile_pool(name="qTnb", bufs=2) as qTnbpool, \
             tc.tile_pool(name="p2q", bufs=3) as p2q:
            kTp = persist.tile([128, H, M], bf16)          # padded kT [d<=72, h, m]
            v16 = persist.tile([128, MC, H * 73], bf16)    # v + den-ones col per head
            eps_sb = persist.tile([128, 1], f32)
            nc.vector.memset(eps_sb, EPS)
            if with_bias:
                ones1 = persist.tile([1, 128], bf16)
                nc.vector.memset(ones1, 1.0)
                qb_sb = persist.tile([1, C], bf16)
                kvb_sb = persist.tile([1, 2 * C], bf16)
                pb_sb = persist.tile([1, C], bf16)
                maskv_sb = persist.tile([1, M], bf16)
                nc.sync.dma_start(out=qb_sb, in_=qb)
                nc.sync.dma_start(out=kvb_sb, in_=kvb)
                nc.sync.dma_start(out=pb_sb, in_=pb)
                nc.sync.dma_start(out=maskv_sb, in_=maskv)
            if uniform_scale is None:
                wk_sb = persist.tile([128, H, 128], bf16)
                nc.sync.dma_start(out=wk_sb.rearrange("p h d -> p (h d)"), in_=wk)

            qTp_nbs = [None] * NB

            # ===== early scope: qw + x-stream live through P1+P2 =====
            with tc.tile_pool(name="early", bufs=1) as early, \
                 tc.tile_pool(name="p2x", bufs=3) as p2x:
                qw_sb = early.tile([128, KC, C], bf16)

                # ================= Phase 1: K/V projection =================
                with tc.tile_pool(name="p1sb", bufs=1) as p1sb, \
                     tc.tile_pool(name="p1small", bufs=2) as p1small, \
                     tc.tile_pool(name="p1k", bufs=2) as p1k, \
                     tc.tile_pool(name="p1psK", bufs=2, space="PSUM") as p1psK, \
                     tc.tile_pool(name="p1psV", bufs=2, space="PSUM") as p1psV:
                    kvw_sb = p1sb.tile([128, KC, 2 * C], bf16)
                    for bo in range(0, 2 * C, 512):
                        bw = min(512, 2 * C - bo)
                        nc.gpsimd.dma_start(out=kvw_sb[:, :, bo:bo + bw],
                                            in_=kvw[:, :, bo:bo + bw])
                    for bo, bw in blocks:
                        nc.gpsimd.dma_start(out=qw_sb[:, :, bo:bo + bw],
                                            in_=qw[:, :, bo:bo + bw])

                    kpend = [None]

                    def k_tail():
                        if kpend[0] is None:
                            return
                        psK, ssqk, mc = kpend[0]
                        kpend[0] = None
                        msl = slice(mc * 128, (mc + 1) * 128)
                        uk = p1small.tile([128, H], f32, tag="uk",
                                           name=f"uk{mc}")
                        invk = p1small.tile([128, H], f32, tag="invk",
                                            name=f"invk{mc}")
                        wkn = p1small.tile([128, H], f32, tag="wkn",
                                           name=f"wkn{mc}")
                        nc.vector.tensor_scalar(uk, ssqk, 1.0 / D, EPS,
                                                op0=ALU.mult, op1=ALU.add)
                        uki = uk.bitcast(mybir.dt.int32)
                        iki = invk.bitcast(mybir.dt.int32)
                        nc.vector.tensor_scalar(
                            iki, uki, 1, None, op0=ALU.logical_shift_right)
                        nc.vector.tensor_scalar(
                            iki, iki, -1, 0x5F3759DF, op0=ALU.mult, op1=ALU.add)
                        for _ in range(2):
                            nc.vector.tensor_tensor(wkn, invk, invk,
                                                    op=ALU.mult)
                            nc.vector.tensor_tensor(wkn, wkn, uk, op=ALU.mult)
                            nc.vector.tensor_scalar(wkn, wkn, -0.5, 1.5,
                                                    op0=ALU.mult, op1=ALU.add)
                            nc.vector.tensor_tensor(invk, invk, wkn,
                                                    op=ALU.mult)
                        # head-padded row layout: head h at cols [128h, 128h+72)
                        k16p = p1k.tile([128, H, 128], bf16, tag="k16",
                                        name=f"k16_{mc}")
                        nc.vector.tensor_tensor(
                            out=k16p[:, :, 0:D],
                            in0=psK[:, 0:C].rearrange("p (h d) -> p h d", h=H),
                            in1=bcast(invk, H, D), op=ALU.mult)
                        if uniform_scale is None:
                            nc.vector.tensor_tensor(
                                out=k16p[:, :, 0:D], in0=k16p[:, :, 0:D],
                                in1=wk_sb[:, :, 0:D], op=ALU.mult)
                        nc.sync.dma_start(
                            out=kTp[:, :, msl],
                            in_=k16p.rearrange("p h d -> p (h d)"),
                            transpose=True)

                    for mc in range(MC):
                        msl = slice(mc * 128, (mc + 1) * 128)
                        condT_sb = p1k.tile([128, KC, 128], bf16, tag="ct",
                                            name=f"ct{mc}")
                        nc.sync.dma_start(out=condT_sb, in_=condT[mc])
                        psK = p1psK.tile([128, 1536], f32, tag="psK",
                                         name=f"psK{mc}")
                        for bo, bw in blocks:
                            for kc in range(KC):
                                nc.tensor.matmul(
                                    psK[:, bo:bo + bw],
                                    condT_sb[:, kc, :],
                                    kvw_sb[:, kc, bo:bo + bw],
                                    start=(kc == 0),
                                    stop=(kc == KC - 1 and not with_bias))
                            if with_bias:
                                nc.tensor.matmul(psK[:, bo:bo + bw],
                                                 maskv_sb[:, msl],
                                                 kvb_sb[:, bo:bo + bw],
                                                 start=False, stop=True)
                        ksq = p1small.tile([128, C], bf16, tag="ksq",
                                           name=f"ksq{mc}")
                        nc.scalar.activation(ksq, psK[:, 0:C], AF.Square)
                        ssqk = p1small.tile([128, H], f32, tag="ssqk",
                                            name=f"ssqk{mc}")
                        nc.vector.tensor_reduce(
                            ssqk, ksq.rearrange("p (h d) -> p h d", h=H),
                            axis=mybir.AxisListType.X, op=ALU.add)
                        k_tail()
                        kpend[0] = (psK, ssqk, mc)

                        for vo, hh in vblocks:
                            vw = hh * D
                            psV = p1psV.tile([128, 504], f32, tag="psV",
                                             name=f"psV{mc}_{vo}")
                            for kc in range(KC):
                                nc.tensor.matmul(
                                    psV[:, 0:vw],
                                    condT_sb[:, kc, :],
                                    kvw_sb[:, kc, C + vo:C + vo + vw],
                                    start=(kc == 0),
                                    stop=(kc == KC - 1 and not with_bias))
                            if with_bias:
                                nc.tensor.matmul(psV[:, 0:vw], maskv_sb[:, msl],
                                                 kvb_sb[:, C + vo:C + vo + vw],
                                                 start=False, stop=True)
                            h0 = vo // D
                            vdst = v16[:, mc, 73 * h0:73 * (h0 + hh)]
                            nc.vector.tensor_copy(
                                out=vdst.rearrange("p (h t) -> p h t",
                                                   h=hh)[:, :, 0:D],
                                in_=psV[:, 0:vw].rearrange("p (h d) -> p h d",
                                                           h=hh))
                        nc.sync.dma_start(
                            out=v16[:, mc, :].rearrange("p (h t) -> p h t",
                                                        h=H)[:, :, 72:73],
                            in_=vones[msl, :])
                    k_tail()

                # ================= Phase 2a: Q projection =================
                # Per tile: matmuls -> ACT copy -> ACT square -> DVE per-head
                # reduce into one shared ssq tile. The rsqrt is batched over
                # 8-tile halves (not per tile), so no ACT queue-head op ever
                # waits on a same-tile cross-engine result and the PE stream
                # never stalls on the rms chain. The first half's scale +
                # transposes (q_finish) overlap the second half's matmuls.
                with tc.tile_pool(name="p2small", bufs=3) as p2small, \
                     tc.tile_pool(name="p2ps", bufs=2, space="PSUM") as p2ps:
                    ssq_all = persist.tile([128, QT, H], f32)
                    inv_all = persist.tile([128, QT, H], f32)
                    sd_all = persist.tile([128, QT, H], f32)
                    rs_scr = persist.tile([128, QT, H], f32)
                    qcps = [None] * QT

                    def q_rsqrt(lo, hi):
                        # DVE-only rsqrt (bit trick + 2 Newton steps): keeps
                        # every ACT queue entry free of DVE dependencies.
                        sl = slice(lo * H, hi * H)
                        src = ssq_all.rearrange("p t h -> p (t h)")[:, sl]
                        u = sd_all.rearrange("p t h -> p (t h)")[:, sl]
                        y = inv_all.rearrange("p t h -> p (t h)")[:, sl]
                        w = rs_scr.rearrange("p t h -> p (t h)")[:, sl]
                        nc.vector.tensor_scalar(u, src, 1.0 / D, EPS,
                                                op0=ALU.mult, op1=ALU.add)
                        ui = u.bitcast(mybir.dt.int32)
                        yi = y.bitcast(mybir.dt.int32)
                        nc.vector.tensor_scalar(
                            yi, ui, 1, None, op0=ALU.logical_shift_right)
                        nc.vector.tensor_scalar(
                            yi, yi, -1, 0x5F3759DF, op0=ALU.mult, op1=ALU.add)
                        for _ in range(2):
                            nc.vector.tensor_tensor(w, y, y, op=ALU.mult)
                            nc.vector.tensor_tensor(w, w, u, op=ALU.mult)
                            nc.vector.tensor_scalar(w, w, -0.5, 1.5,
                                                    op0=ALU.mult, op1=ALU.add)
                            nc.vector.tensor_tensor(y, y, w, op=ALU.mult)

                    def q_finish_tile(qt):
                        nb, j = qt // 4, qt % 4
                        if j == 0:
                            qTp_nbs[nb] = qTnbpool.tile(
                                [128, H, 512], bf16, tag="qTp",
                                name=f"qTp{nb}")
                        q16p = p2q.tile([128, H, 128], bf16, tag="q16",
                                        name=f"q16_{qt}")
                        nc.vector.tensor_tensor(
                            out=q16p[:, :, 0:D],
                            in0=qcps[qt].rearrange("p (h d) -> p h d", h=H),
                            in1=bcast(inv_all[:, qt, :], H, D),
                            op=ALU.mult)
                        nc.sync.dma_start(
                            out=qTp_nbs[nb][:, :, j * 128:(j + 1) * 128],
                            in_=q16p.rearrange("p h d -> p (h d)"),
                            transpose=True)

                    def q_finish(nb):
                        for j in range(4):
                            q_finish_tile(nb * 4 + j)

                    for qt in range(QT):
                        xt = p2x.tile([128, KC, 128], bf16, tag="xt",
                                      name=f"xt{qt}")
                        nc.gpsimd.dma_start(out=xt, in_=xTt[qt])
                        psQ = p2ps.tile([128, 1536], f32, tag="psQ",
                                        name=f"psQ{qt}")
                        for bo, bw in blocks:
                            for kc in range(KC):
                                nc.tensor.matmul(
                                    psQ[:, bo:bo + bw], xt[:, kc, :],
                                    qw_sb[:, kc, bo:bo + bw],
                                    start=(kc == 0),
                                    stop=(kc == KC - 1 and not with_bias))
                            if with_bias:
                                nc.tensor.matmul(psQ[:, bo:bo + bw], ones1,
                                                 qb_sb[:, bo:bo + bw],
                                                 start=False, stop=True)
                        qcp = persist.tile([128, C], bf16, name=f"qcp{qt}")
                        qcps[qt] = qcp
                        nc.scalar.activation(qcp, psQ[:, 0:C], AF.Copy)
                        qsq = p2small.tile([128, C], bf16, tag="qsq",
                                           name=f"qsq{qt}")
                        nc.scalar.activation(qsq, qcp, AF.Square)
                        nc.vector.tensor_reduce(
                            ssq_all[:, qt, :],
                            qsq.rearrange("p (h d) -> p h d", h=H),
                            axis=mybir.AxisListType.X, op=ALU.add)
                        if qt == 8:
                            q_rsqrt(0, 8)
                        if qt >= 9:
                            q_finish_tile(qt - 9)
                    q_rsqrt(8, QT)
                    q_finish_tile(7)

            # ===== Phases 3+4: attention + output projection (interleaved) =====
            with tc.tile_pool(name="p3exp", bufs=HB + 2) as p3exp, \
                 tc.tile_pool(name="p3attn", bufs=6) as p3attn, \
                 tc.tile_pool(name="p3small", bufs=4) as p3small, \
                 tc.tile_pool(name="attnTnb", bufs=2) as attnTnb, \
                 tc.tile_pool(name="p4o", bufs=2) as p4o, \
                 tc.tile_pool(name="p3psS", bufs=2, space="PSUM") as p3psS, \
                 tc.tile_pool(name="p3psPV", bufs=2, space="PSUM") as p3psPV, \
                 tc.tile_pool(name="pwpool", bufs=1) as pwpool, \
                 tc.tile_pool(name="p4ps", bufs=2, space="PSUM") as p4ps:
                pw_sb = pwpool.tile([128, KC, C], bf16)
                for bo, bw in blocks:
                    nc.gpsimd.dma_start(out=pw_sb[:, :, bo:bo + bw],
                                        in_=pw[:, :, bo:bo + bw])

                attnTs = [None] * NB
                prev_proj = [None]   # chain proj accumulation groups only

                def proj_qt_blocks(pnb, j):
                    """Yield per-block emitters for one 128-row output tile,
                    so proj matmuls can interleave between score emissions."""
                    qt = pnb * 4 + j
                    so = p4o.tile([128, C], f32, tag="so", name=f"so_{qt}")

                    def emit_block(bo, bw, is_last):
                        psP = p4ps.tile([128, 512], f32, tag="psP",
                                        name=f"psP_{qt}_{bo}")
                        first = last = None
                        for kc in range(KC):
                            mm = nc.tensor.matmul(
                                psP[:, 0:bw],
                                attnTs[pnb][:, kc, j * 128:(j + 1) * 128],
                                pw_sb[:, kc, bo:bo + bw],
                                start=(kc == 0),
                                stop=(kc == KC - 1 and not with_bias))
                            if first is None:
                                first = mm
                            last = mm
                        if with_bias:
                            last = nc.tensor.matmul(psP[:, 0:bw], ones1,
                                                    pb_sb[:, bo:bo + bw],
                                                    start=False, stop=True)
                        if prev_proj[0] is not None:
                            tile.add_dep_helper(first.ins, prev_proj[0].ins,
                                                sync=False,
                                                reason="proj psum group order")
                        prev_proj[0] = last
                        nc.vector.tensor_copy(so[:, bo:bo + bw], psP[:, 0:bw])
                        if is_last:
                            nc.sync.dma_start(
                                out=out[qt * 128:(qt + 1) * 128, :], in_=so)

                    return [lambda bo=bo, bw=bw, lastb=(i == len(blocks) - 1):
                            emit_block(bo, bw, lastb)
                            for i, (bo, bw) in enumerate(blocks)]

                def proj_qt(pnb, j):
                    for emit in proj_qt_blocks(pnb, j):
                        emit()

                for nb in range(NB):
                    if nb + 2 < NB:
                        q_finish(nb + 2)
                    qTp_nb = qTp_nbs[nb]
                    attnT = attnTnb.tile([128, KC, 512], bf16, tag="attnT",
                                         name=f"attnT{nb}")
                    attnTs[nb] = attnT
                    attnN = [p3attn.tile([128, C], bf16, tag="attnN",
                                         name=f"attnN_{nb}_{j}")
                             for j in range(4)]
                    for hg in range(NHG):
                        pblocks = (proj_qt_blocks(nb - 1, hg) if nb > 0
                                   else [None] * 3)
                        expts = []
                        for hl in range(HB):
                            h = hg * HB + hl
                            expt = p3exp.tile([128, 2048], bf16, tag="expt",
                                              name=f"expt_{nb}_{h}")
                            for half in range(2):
                                psS = p3psS.tile([128, 1024], f32, tag="psS",
                                                 name=f"psS_{nb}_{h}_{half}")
                                for mcl in range(2):
                                    mc = half * 2 + mcl
                                    nc.tensor.matmul(
                                        psS[:, mcl * 512:(mcl + 1) * 512],
                                        kTp[0:D, h, mc * 128:(mc + 1) * 128],
                                        qTp_nb[0:D, h, :],
                                        start=True, stop=True)
                                nc.scalar.activation(
                                    expt[:, half * 1024:(half + 1) * 1024],
                                    psS, AF.Exp, scale=exp_scale)
                            expts.append(expt)
                            # fill the exp-wait PE bubble with a proj block
                            if hl >= 1 and pblocks[hl - 1] is not None:
                                pblocks[hl - 1]()
                        for j in range(4):
                            # HB heads share one PSUM bank; accumulation groups
                            # are chained in emission order (start=True clears
                            # the whole bank's has_written bits).
                            psPV = p3psPV.tile([128, 512], f32, tag="psPV",
                                               name=f"psPV_{nb}_{hg}_{j}")
                            prev_last = None
                            for hl in range(HB):
                                h = hg * HB + hl
                                first = last = None
                                for mc in range(MC):
                                    mm = nc.tensor.matmul(
                                        psPV[:, hl * 73:(hl + 1) * 73],
                                        expts[hl][:, mc * 512 + j * 128:
                                                  mc * 512 + (j + 1) * 128],
                                        v16[:, mc, 73 * h:73 * h + 73],
                                        start=(mc == 0), stop=(mc == MC - 1))
                                    if first is None:
                                        first = mm
                                    last = mm
                                if prev_last is not None:
                                    tile.add_dep_helper(
                                        first.ins, prev_last.ins, sync=False,
                                        reason="psum-bank accum group order")
                                prev_last = last
                            dens = p3small.tile([128, HB], f32, tag="dens",
                                                name=f"dens_{nb}_{hg}_{j}")
                            pv3 = bass.AP(
                                tensor=psPV.tensor, offset=psPV.offset,
                                ap=[psPV.ap[0], [73, HB], [1, 73]])
                            nc.vector.tensor_copy(out=dens, in_=pv3[:, :, 72])
                            rec = p3small.tile([128, HB], f32, tag="rec",
                                               name=f"rec_{nb}_{hg}_{j}")
                            nc.vector.reciprocal(rec, dens)
                            nc.vector.tensor_tensor(
                                out=attnN[j][:, hg * HB * D:(hg + 1) * HB * D]
                                    .rearrange("p (h d) -> p h d", h=HB),
                                in0=pv3[:, :, 0:D],
                                in1=bcast(rec, HB, D), op=ALU.mult)
                    for j in range(4):
                        nc.sync.dma_start(
                            out=attnT[:, :, j * 128:(j + 1) * 128],
                            in_=attnN[j], transpose=True)
                    if nb == NB - 1:
                        for j in range(4):
                            proj_qt(nb, j)

    nc.compile()
    _BUILD_CACHE[key] = nc
    return nc


def kernel(x, cond, kv_seqlen, q_w, q_b, kv_w, kv_b, proj_w, proj_b, qn_w, kn_w):
    x = np.asarray(x); cond = np.asarray(cond)
    kv_seqlen = np.asarray(kv_seqlen)
    q_w = np.asarray(q_w, np.float32); q_b = np.asarray(q_b, np.float32)
    kv_w = np.asarray(kv_w, np.float32); kv_b = np.asarray(kv_b, np.float32)
    proj_w = np.asarray(proj_w, np.float32); proj_b = np.asarray(proj_b, np.float32)
    qn_w = np.asarray(qn_w, np.float32); kn_w = np.asarray(kn_w, np.float32)

    with_bias = bool(np.any(q_b) or np.any(kv_b) or np.any(proj_b))
    qk = (qn_w * kn_w).astype(np.float64)
    if np.all(qk == qk[0]):
        uniform_scale = float(qk[0]) / float(np.sqrt(D))
    else:
        uniform_scale = None
    key = (with_bias, uniform_scale)
    nc = _build(with_bias, uniform_scale, key)

    def blocked_w(w):  # [C, dout] -> [128, KC, dout]
        return np.ascontiguousarray(
            w.reshape(KC, 128, -1).transpose(1, 0, 2)).astype(BF16)

    qwb = blocked_w(q_w)
    kvwb = blocked_w(kv_w)
    pwb = blocked_w(proj_w)
    if uniform_scale is None:
        wk_pad = np.zeros((H, 128), np.float32)
        wk_pad[:, 0:D] = (qn_w * kn_w)[None, :] / np.sqrt(D)
        wkb = np.ascontiguousarray(
            np.broadcast_to(wk_pad.reshape(1, H * 128),
                            (128, H * 128))).astype(BF16)

    in_maps = []
    for core in range(NCORES):
        b, ns = core // 2, (core % 2) * NL
        A = x[b, ns:ns + NL, :].astype(np.float32)
        xtt = np.ascontiguousarray(
            A.reshape(QT, 128, KC, 128).transpose(0, 3, 2, 1)).astype(BF16)
        sl = int(kv_seqlen[b])
        ct = cond[b].astype(np.float32).T.copy()       # [C, M]
        ct[:, sl:] = 0.0
        ctb = np.ascontiguousarray(
            ct.reshape(KC, 128, MC, 128).transpose(2, 1, 0, 3)).astype(BF16)
        valid = (np.arange(M) < sl)
        vob = np.ascontiguousarray(
            np.repeat(valid[:, None], H, axis=1)).astype(BF16)
        m = {"xTt": xtt, "condT": ctb, "qw": qwb, "kvw": kvwb, "pw": pwb,
             "vones": vob}
        if uniform_scale is None:
            m["wk"] = wkb
        if with_bias:
            m["qb"] = q_b[None, :].astype(BF16)
            m["kvb"] = kv_b[None, :].astype(BF16)
            m["pb"] = proj_b[None, :].astype(BF16)
            m["maskv"] = valid[None, :].astype(BF16)
        in_maps.append(m)

    res = run_bass_kernel_spmd(nc, in_maps, core_ids=list(range(NCORES)))
    kernel._last_results = res

    out = np.empty((B, N, C), np.float32)
    for core in range(NCORES):
        b, ns = core // 2, (core % 2) * NL
        out[b, ns:ns + NL, :] = res.results[core]["out"]
    return out
